# revision 19
# baseline (speedup 1.0000x reference)
"""DLRM forward (embedding gather + tiny MLPs) as a Bass/Tile kernel on 8 trn2 cores.

Sharding: data-parallel over the batch (tables replicated in each core's HBM —
total gather traffic is the same as model-parallel but needs no collectives).
Each core computes 2048 rows end-to-end and returns [1, 2048] sigmoid outputs.

v7 final (v6 42.2µs, v5 44.3µs, v4 46.9µs, v3 62.5µs, v1 68.7µs, v0 110.9µs):
  - fp8e4m3 tables (x16 scale, numpy-validated: adds only ~1e-4 rel err)
    bit-packed as u16 pairs -> gather traffic halves to 64B/row AND one
    bf16-typed PE transpose moves two fp8 k-values per lane (28 transposes
    + 4 casts per 512-sample group instead of 52 + 13). Our fp8 bytes can
    never form bf16 NaN/Inf patterns, so the transposes are bit-safe.
  - the pair-per-partition layout feeds DoubleRow fp8 matmuls directly
    (256 k contracted per 512-column pass, 0.5 cycles/row).
  - all small weights (identities, bias-augmented MLP weights, DoubleRow-
    packed tw1, scaled tw1-dense tail) travel in ONE blob DMA: the
    prologue stalls on instruction fetch + const DMAs, and serial ~700ns
    DMA issues were the critical path.
  - warm-up matmuls during the first-gather wait engage the PE clock
    governor (HAM ignores transpose-mode work; without them the first
    ~15µs ran at half clock).
  - bottom MLP (fp16, biases folded via ones-rows / a host-side ones
    column in dense_x) runs two groups ahead of the top-MLP stream;
    group tails (relu+tw2+sigmoid+y-DMA) are deferred into the next
    group; the last group's tail is split in halves to pipeline the
    drain.
"""

import numpy as np
import ml_dtypes

import concourse.bass as bass
import concourse.mybir as mybir
import concourse.tile as tile
from concourse import bacc

P = 128

N_CORES = 8
B = 16384
F = 26
D = 64
DENSE = 13
DENSE_A = DENSE + 1           # host-appended ones column
CARD = 100000
H_BOT = 8
H_TOP = 16

f32 = mybir.dt.float32
i32 = mybir.dt.int32
fp16 = mybir.dt.float16
fp8 = mybir.dt.float8e4
u16 = mybir.dt.uint16
bf16 = mybir.dt.bfloat16

B_LOC = B // N_CORES          # 2048
K_EMB = F * D                 # 1664
GROUP = 512
TPG = GROUP // P              # 4
N_G = B_LOC // GROUP          # 4
N_T = B_LOC // P              # 16

W_T = F * D // 2              # 832 u16 per sample-tile row
CW = [128] * 6 + [64]         # u16 chunk widths (pairs of fp8 k-values)

FP8_SCALE = 16.0              # tables and tw1 each scaled by this
N_WARM = 10                   # PE warm-up matmuls during the gather wait
BLOB_W = 460                  # fp16 columns in the packed weight blob


def build_kernel():
    nc = bacc.Bacc("TRN2", target_bir_lowering=False)
    comb_d = nc.dram_tensor("sparse_x", [P, N_T * F], i32, kind="ExternalInput")
    dense_d = nc.dram_tensor("dense_x", [P, N_T * DENSE_A], fp16, kind="ExternalInput")
    tables_d = nc.dram_tensor("tables", [F * CARD, D // 2], u16, kind="ExternalInput")
    blob_d = nc.dram_tensor("blob", [P, BLOB_W], fp16, kind="ExternalInput")
    y_d = nc.dram_tensor("y", [1, B_LOC], f32, kind="ExternalOutput")

    with tile.TileContext(nc) as tc:
        with (
            tc.tile_pool(name="const", bufs=1) as cpool,
            tc.tile_pool(name="embt", bufs=3) as embtp,
            tc.tile_pool(name="small", bufs=3) as smallp,
            tc.tile_pool(name="ptr", bufs=3, space="PSUM") as ptrp,
            tc.tile_pool(name="po1", bufs=2, space="PSUM") as po1p,
            tc.tile_pool(name="pwarm", bufs=1, space="PSUM") as pwarmp,
            tc.tile_pool(name="psmall", bufs=2, space="PSUM") as psmallp,
        ):
            # ---- index DMA first ----
            comb_sb = cpool.tile([P, N_T * F], i32)
            nc.sync.dma_start(out=comb_sb[:], in_=comb_d[:, :])

            # ---- gather dispatches (gpsimd only): group 0 split for latency ----
            embs = [cpool.tile([P, TPG * W_T], u16, name=f"emb{g}") for g in range(N_G)]
            spans = [(0, 0, 2 * F), (0, 2 * F, 4 * F)] + [
                (g, 0, TPG * F) for g in range(1, N_G)
            ]
            for g, f0, f1 in spans:
                nc.gpsimd.indirect_dma_start(
                    out=embs[g][:, f0 * (D // 2) : f1 * (D // 2)],
                    out_offset=None,
                    in_=tables_d[:, :],
                    in_offset=bass.IndirectOffsetOnAxis(
                        ap=comb_sb[:, g * TPG * F + f0 : g * TPG * F + f1], axis=0
                    ),
                )

            # ---- all small weights arrive in ONE blob DMA (the prologue
            # has a barrier that waits for every const DMA; v6 lost ~2.5µs
            # to seven serial ~700ns DMA issues) ----
            dense_sb = cpool.tile([P, N_T * DENSE_A], fp16)
            nc.sync.dma_start(out=dense_sb[:], in_=dense_d[:, :])
            blob_sb = cpool.tile([P, BLOB_W], fp16)
            nc.scalar.dma_start(out=blob_sb[:], in_=blob_d[:, :])
            ident = blob_sb[:, 0:128]
            identu = blob_sb[:, 128:256].bitcast(bf16)
            tw1dr_sb = blob_sb[:, 256:368].bitcast(fp8)     # 112 fp16 = 224 fp8
            w1a_sb = blob_sb[0:DENSE_A, 368:376]
            w2a_sb = blob_sb[0 : H_BOT + 1, 376:440]
            tw1da_sb = blob_sb[0 : D + 1, 440:456]
            tw2_sb = blob_sb[0:H_TOP, 456:457]
            tb2_sb = blob_sb[0:1, 458:460].bitcast(f32)

            y_row = cpool.tile([1, B_LOC], f32)
            dta = [cpool.tile([D + 1, GROUP], fp16, name=f"dta{g}") for g in range(N_G)]
            ha2 = [cpool.tile([H_BOT + 1, GROUP], fp16, name=f"ha{i}") for i in range(2)]
            for t in ha2 + dta[:2]:
                nc.vector.memset(t[:], 1.0)
            for t in dta[2:]:
                nc.gpsimd.memset(t[:], 1.0)

            # ---- dense bottom-MLP pieces (biases folded via ones-rows;
            # dxt's ones row comes in with the host data) ----
            def dense_tr(g):
                pdx_t = psmallp.tile([DENSE_A, GROUP], fp16, tag="psmall")
                for j in range(TPG):
                    t = g * TPG + j
                    nc.tensor.transpose(
                        out=pdx_t[:, bass.ts(j, P)],
                        in_=dense_sb[:, bass.ts(t, DENSE_A)],
                        identity=ident,
                    )
                return pdx_t

            def dense_mm1(g, pdx_t):
                dxt = smallp.tile([DENSE_A, GROUP], fp16, tag="dxt")
                nc.vector.tensor_copy(out=dxt[:], in_=pdx_t[:])
                ph = psmallp.tile([H_BOT, GROUP], f32, tag="psmall")
                nc.tensor.matmul(out=ph[:], lhsT=w1a_sb, rhs=dxt[:], start=True, stop=True)
                return ph

            def dense_mm2(g, ph):
                ha = ha2[g % 2]
                nc.scalar.activation(
                    out=ha[0:H_BOT, :], in_=ph[:],
                    func=mybir.ActivationFunctionType.Relu,
                )
                pd = psmallp.tile([D, GROUP], f32, tag="psmall")
                nc.tensor.matmul(out=pd[:], lhsT=w2a_sb, rhs=ha[:], start=True, stop=True)
                nc.scalar.activation(
                    out=dta[g][0:D, :], in_=pd[:],
                    func=mybir.ActivationFunctionType.Copy,
                )

            # group 0+1 dense runs up front, under the first gather; the PE
            # warm-up matmuls engage the HAM clock boost (transposes don't
            # count as PE-busy for it)
            pwarm = pwarmp.tile([H_TOP, GROUP], f32, tag="pwarm")
            pdx0 = dense_tr(0)
            ph0 = dense_mm1(0, pdx0)
            pdx1 = dense_tr(1)
            dense_mm2(0, ph0)
            ph1 = dense_mm1(1, pdx1)
            dense_mm2(1, ph1)
            for _ in range(N_WARM):
                nc.tensor.matmul(
                    out=pwarm[:, 0:224], lhsT=ident[:, 0:H_TOP], rhs=dense_sb[:, 0:224],
                    start=True, stop=True,
                )

            # ---- fp8-pair transposes + DoubleRow top-MLP accumulation ----
            tail = []  # deferred (o1 relu, tw2 matmul, sigmoid) of prev group

            def flush_tail():
                while tail:
                    tail.pop(0)()

            for g in range(N_G):
                po1 = po1p.tile([H_TOP, GROUP], f32, tag="po1")
                mms = []
                casted = []

                def emit_tchunks(dc, g=g, mms=mms, casted=casted, po1=po1):
                    # transposes for two k-chunks into one PSUM tile
                    cs = [2 * dc] + ([2 * dc + 1] if 2 * dc + 1 < 7 else [])
                    ptr_t = ptrp.tile([P, 2 * GROUP], bf16, tag="ptr")
                    for ci, c in enumerate(cs):
                        cw = CW[c]
                        for j in range(TPG):
                            o = j * W_T + c * 128
                            nc.tensor.transpose(
                                out=ptr_t[0:cw, ci * GROUP + j * P : ci * GROUP + (j + 1) * P],
                                in_=embs[g][:, o : o + cw].bitcast(bf16),
                                identity=identu,
                            )
                    casted.append((dc, cs, ptr_t))

                def emit_cast(g=g, mms=mms, casted=casted, po1=po1):
                    dc, cs, ptr_t = casted.pop(0)
                    embt = embtp.tile([P, 2 * GROUP], bf16, tag="embt")
                    wid = len(cs) * GROUP
                    if dc % 2 == 0:
                        nc.vector.tensor_copy(out=embt[:, 0:wid], in_=ptr_t[:, 0:wid])
                    else:
                        nc.scalar.activation(
                            out=embt[:, 0:wid], in_=ptr_t[:, 0:wid],
                            func=mybir.ActivationFunctionType.Copy,
                        )
                    for ci, c in enumerate(cs):
                        def mm(c=c, ci=ci, embt=embt):
                            cw = CW[c]
                            lhsT = tw1dr_sb[0:cw, c * 32 : (c + 1) * 32].rearrange(
                                "p (i m) -> p i m", i=2
                            )
                            rhs = embt[0:cw, bass.ts(ci, GROUP)].bitcast(fp8).rearrange(
                                "p (n i) -> p i n", i=2
                            )
                            nc.tensor.matmul(
                                out=po1[:], lhsT=lhsT, rhs=rhs,
                                start=(c == 0), stop=False,
                                perf_mode=mybir.MatmulPerfMode.DoubleRow,
                            )
                        mms.append(mm)

                # transpose phase (casts trail by one dchunk), then matmul phase
                emit_tchunks(0)
                emit_cast()
                emit_tchunks(1)
                if g < N_G - 2:
                    pdx_t = dense_tr(g + 2)
                emit_cast()
                flush_tail()
                emit_tchunks(2)
                emit_cast()
                mms.pop(0)()                      # MM(0)
                mms.pop(0)()                      # MM(1)
                if g < N_G - 2:
                    ph = dense_mm1(g + 2, pdx_t)
                emit_tchunks(3)                   # single chunk 6
                emit_cast()
                mms.pop(0)()                      # MM(2)
                mms.pop(0)()                      # MM(3)
                if g < N_G - 2:
                    dense_mm2(g + 2, ph)
                mms.pop(0)()                      # MM(4)
                mms.pop(0)()                      # MM(5)
                mms.pop(0)()                      # MM(6)
                nc.tensor.matmul(
                    out=po1[:], lhsT=tw1da_sb, rhs=dta[g][:], start=False, stop=True
                )

                def make_tail(g=g, po1=po1):
                    halves = 2 if g == N_G - 1 else 1
                    def run():
                        o1 = smallp.tile([H_TOP, GROUP], fp16, tag="o1")
                        w = GROUP // halves
                        plgs = []
                        for h in range(halves):
                            nc.vector.tensor_scalar(
                                out=o1[:, h * w : (h + 1) * w],
                                in0=po1[:, h * w : (h + 1) * w],
                                scalar1=1.0 / (FP8_SCALE * FP8_SCALE), scalar2=0.0,
                                op0=mybir.AluOpType.mult, op1=mybir.AluOpType.max,
                            )
                            plg = psmallp.tile([1, GROUP], f32, tag="psmall")
                            nc.tensor.matmul(
                                out=plg[:, 0:w], lhsT=tw2_sb,
                                rhs=o1[:, h * w : (h + 1) * w], start=True, stop=True,
                            )
                            plgs.append(plg)
                        for h, plg in enumerate(plgs):
                            nc.scalar.activation(
                                out=y_row[:, g * GROUP + h * w : g * GROUP + (h + 1) * w],
                                in_=plg[:, 0:w],
                                func=mybir.ActivationFunctionType.Sigmoid,
                                bias=tb2_sb,
                            )
                        nc.sync.dma_start(
                            out=y_d[:, bass.ts(g, GROUP)],
                            in_=y_row[:, bass.ts(g, GROUP)],
                        )
                    return run

                tail.append(make_tail())
            flush_tail()

    nc.compile()
    return nc


_NC_CACHE = {}


def _get_nc():
    if "nc" not in _NC_CACHE:
        _NC_CACHE["nc"] = build_kernel()
    return _NC_CACHE["nc"]


FP8_NP = ml_dtypes.float8_e4m3  # what mybir.dt.float8e4 maps to


def make_in_maps(dense_x, sparse_x, tables, w1, b1, w2, b2, tw1, tb1, tw2, tb2):
    s = FP8_SCALE
    t8 = (np.asarray(tables, np.float32).reshape(F * CARD, D) * s).astype(FP8_NP)
    tables_u16 = np.ascontiguousarray(t8).view(np.uint16)  # [V, 32]
    comb = np.asarray(sparse_x).astype(np.int32) + (
        np.arange(F, dtype=np.int32) * CARD
    )[None, :]
    dense_f = np.asarray(dense_x).astype(np.float16)
    dense_aug = np.concatenate(
        [dense_f, np.ones((dense_f.shape[0], 1), np.float16)], axis=1
    )  # [B, 14] with ones column
    tw1 = np.asarray(tw1, np.float32)
    # tw1dr[p, c*32 + i*16 + m] = fp8(s * tw1[c*256 + 2p + i, m])
    tw1dr = np.zeros((P, 7 * 2 * H_TOP), dtype=FP8_NP)
    for c in range(7):
        rows = 2 * CW[c]
        blk = (tw1[c * 256 : c * 256 + rows] * s).astype(FP8_NP)  # [rows, 16]
        blk = blk.reshape(CW[c], 2, H_TOP).reshape(CW[c], 2 * H_TOP)
        tw1dr[0 : CW[c], c * 32 : (c + 1) * 32] = blk
    tw1da = np.vstack(
        [tw1[K_EMB : K_EMB + D], np.asarray(tb1, np.float32)[None, :]]
    ) * (s * s)
    blob = np.zeros((P, BLOB_W), dtype=np.float16)
    blob[:, 0:128] = np.eye(P, dtype=np.float16)
    blob[:, 128:256] = np.eye(P, dtype=ml_dtypes.bfloat16).view(np.uint16).view(np.float16)
    blob[:, 256:368] = tw1dr.view(np.uint16).view(np.float16)  # 224 fp8 -> 112 fp16
    blob[0:DENSE_A, 368:376] = np.vstack(
        [np.asarray(w1, np.float32), np.asarray(b1, np.float32)[None, :]]
    ).astype(np.float16)
    blob[0 : H_BOT + 1, 376:440] = np.vstack(
        [np.asarray(w2, np.float32), np.asarray(b2, np.float32)[None, :]]
    ).astype(np.float16)
    blob[0 : D + 1, 440:456] = tw1da.astype(np.float16)
    blob[0:H_TOP, 456:457] = np.asarray(tw2, np.float32).astype(np.float16)
    blob[0:1, 458:460] = (
        np.asarray(tb2, np.float32).reshape(1, 1).view(np.float16)
    )
    shared = {
        "tables": tables_u16,
        "blob": blob,
    }
    in_maps = []
    for c in range(N_CORES):
        m = dict(shared)
        # host pre-transpose: [p, (t f)] so the device DMA is contiguous
        dl = dense_aug[c * B_LOC : (c + 1) * B_LOC]
        m["dense_x"] = np.ascontiguousarray(
            dl.reshape(N_T, P, DENSE_A).transpose(1, 0, 2).reshape(P, N_T * DENSE_A)
        )
        cl = comb[c * B_LOC : (c + 1) * B_LOC]
        m["sparse_x"] = np.ascontiguousarray(
            cl.reshape(N_T, P, F).transpose(1, 0, 2).reshape(P, N_T * F)
        )
        in_maps.append(m)
    return in_maps


def kernel(**inputs):
    from concourse.bass_utils import run_bass_kernel_spmd

    nc = _get_nc()
    in_maps = make_in_maps(**inputs)
    res = run_bass_kernel_spmd(nc, in_maps, core_ids=list(range(N_CORES)))
    out = np.concatenate([r["y"].reshape(-1) for r in res.results])
    return out.reshape(B, 1).astype(np.float32)


# revision 20
# speedup vs baseline: 1.0501x; 1.0501x over previous
"""DLRM forward (embedding gather + tiny MLPs) as a Bass/Tile kernel on 8 trn2 cores.

Sharding: data-parallel over the batch (tables replicated in each core's HBM —
total gather traffic is the same as model-parallel but needs no collectives).
Each core computes 2048 rows end-to-end and returns [1, 2048] sigmoid outputs.

v7 final (v6 42.2µs, v5 44.3µs, v4 46.9µs, v3 62.5µs, v1 68.7µs, v0 110.9µs):
  - fp8e4m3 tables (x16 scale, numpy-validated: adds only ~1e-4 rel err)
    bit-packed as u16 pairs -> gather traffic halves to 64B/row AND one
    bf16-typed PE transpose moves two fp8 k-values per lane (28 transposes
    + 4 casts per 512-sample group instead of 52 + 13). Our fp8 bytes can
    never form bf16 NaN/Inf patterns, so the transposes are bit-safe.
  - the pair-per-partition layout feeds DoubleRow fp8 matmuls directly
    (256 k contracted per 512-column pass, 0.5 cycles/row).
  - all small weights (identities, bias-augmented MLP weights, DoubleRow-
    packed tw1, scaled tw1-dense tail) travel in ONE blob DMA: the
    prologue stalls on instruction fetch + const DMAs, and serial ~700ns
    DMA issues were the critical path.
  - warm-up matmuls during the first-gather wait engage the PE clock
    governor (HAM ignores transpose-mode work; without them the first
    ~15µs ran at half clock).
  - bottom MLP (fp16, biases folded via ones-rows / a host-side ones
    column in dense_x) runs two groups ahead of the top-MLP stream;
    group tails (relu+tw2+sigmoid+y-DMA) are deferred into the next
    group; the last group's tail is split in halves to pipeline the
    drain.
"""

import numpy as np
import ml_dtypes

import concourse.bass as bass
import concourse.mybir as mybir
import concourse.tile as tile
from concourse import bacc

P = 128

N_CORES = 8
B = 16384
F = 26
D = 64
DENSE = 13
DENSE_A = DENSE + 1           # host-appended ones column
CARD = 100000
H_BOT = 8
H_TOP = 16

f32 = mybir.dt.float32
i32 = mybir.dt.int32
fp16 = mybir.dt.float16
fp8 = mybir.dt.float8e4
u16 = mybir.dt.uint16
bf16 = mybir.dt.bfloat16

B_LOC = B // N_CORES          # 2048
K_EMB = F * D                 # 1664
GROUP = 512
TPG = GROUP // P              # 4
N_G = B_LOC // GROUP          # 4
N_T = B_LOC // P              # 16

W_T = F * D // 2              # 832 u16 per sample-tile row
CW = [128] * 6 + [64]         # u16 chunk widths (pairs of fp8 k-values)

FP8_SCALE = 16.0              # tables and tw1 each scaled by this
N_WARM = 10                   # PE warm-up matmuls during the gather wait
BLOB_W = 460                  # fp16 columns in the packed weight blob


def build_kernel():
    nc = bacc.Bacc("TRN2", target_bir_lowering=False)
    comb_d = nc.dram_tensor("sparse_x", [P, N_T * F], i32, kind="ExternalInput")
    dense_d = nc.dram_tensor("dense_x", [P, N_T * DENSE_A], fp16, kind="ExternalInput")
    tables_d = nc.dram_tensor("tables", [F * CARD, D // 2], u16, kind="ExternalInput")
    blob_d = nc.dram_tensor("blob", [P, BLOB_W], fp16, kind="ExternalInput")
    y_d = nc.dram_tensor("y", [1, B_LOC], f32, kind="ExternalOutput")

    with tile.TileContext(nc) as tc:
        with (
            tc.tile_pool(name="const", bufs=1) as cpool,
            tc.tile_pool(name="embt", bufs=3) as embtp,
            tc.tile_pool(name="small", bufs=3) as smallp,
            tc.tile_pool(name="ptr", bufs=3, space="PSUM") as ptrp,
            tc.tile_pool(name="po1", bufs=2, space="PSUM") as po1p,
            tc.tile_pool(name="pwarm", bufs=1, space="PSUM") as pwarmp,
            tc.tile_pool(name="psmall", bufs=2, space="PSUM") as psmallp,
        ):
            # ---- index DMA first ----
            comb_sb = cpool.tile([P, N_T * F], i32)
            nc.sync.dma_start(out=comb_sb[:], in_=comb_d[:, :])

            # ---- gather dispatches (gpsimd only): group 0 split for latency ----
            embs = [cpool.tile([P, TPG * W_T], u16, name=f"emb{g}") for g in range(N_G)]
            spans = [(0, 0, 2 * F), (0, 2 * F, 4 * F)] + [
                (g, 0, TPG * F) for g in range(1, N_G)
            ]
            for g, f0, f1 in spans:
                nc.gpsimd.indirect_dma_start(
                    out=embs[g][:, f0 * (D // 2) : f1 * (D // 2)],
                    out_offset=None,
                    in_=tables_d[:, :],
                    in_offset=bass.IndirectOffsetOnAxis(
                        ap=comb_sb[:, g * TPG * F + f0 : g * TPG * F + f1], axis=0
                    ),
                )

            # ---- all small weights arrive in ONE blob DMA (the prologue
            # has a barrier that waits for every const DMA; v6 lost ~2.5µs
            # to seven serial ~700ns DMA issues) ----
            dense_sb = cpool.tile([P, N_T * DENSE_A], fp16)
            nc.sync.dma_start(out=dense_sb[:], in_=dense_d[:, :])
            blob_sb = cpool.tile([P, BLOB_W], fp16)
            nc.scalar.dma_start(out=blob_sb[:], in_=blob_d[:, :])
            ident = blob_sb[:, 0:128]
            identu = blob_sb[:, 128:256].bitcast(bf16)
            tw1dr_sb = blob_sb[:, 256:368].bitcast(fp8)     # 112 fp16 = 224 fp8
            w1a_sb = blob_sb[0:DENSE_A, 368:376]
            w2a_sb = blob_sb[0 : H_BOT + 1, 376:440]
            tw1da_sb = blob_sb[0 : D + 1, 440:456]
            tw2_sb = blob_sb[0:H_TOP, 456:457]
            tb2_sb = blob_sb[0:1, 458:460].bitcast(f32)

            # PE warm-up runs before anything lands from HBM: operands come
            # from a memset-only tile, so the PE starts (and its clock ramps)
            # as soon as its instruction stream is fetched
            wtile = cpool.tile([P, 224], fp16)
            nc.vector.memset(wtile[:], 1.0)
            pwarm = pwarmp.tile([H_TOP, GROUP], f32, tag="pwarm")
            for _ in range(N_WARM):
                nc.tensor.matmul(
                    out=pwarm[:, 0:224], lhsT=wtile[:, 0:H_TOP], rhs=wtile[:],
                    start=True, stop=True,
                )

            y_row = cpool.tile([1, B_LOC], f32)
            dta = [cpool.tile([D + 1, GROUP], fp16, name=f"dta{g}") for g in range(N_G)]
            ha2 = [cpool.tile([H_BOT + 1, GROUP], fp16, name=f"ha{i}") for i in range(2)]
            for t in ha2:
                nc.vector.memset(t[:], 1.0)
            for t in dta:
                nc.gpsimd.memset(t[:], 1.0)

            # ---- dense bottom-MLP pieces (biases folded via ones-rows;
            # dxt's ones row comes in with the host data) ----
            def dense_tr(g):
                pdx_t = psmallp.tile([DENSE_A, GROUP], fp16, tag="psmall")
                for j in range(TPG):
                    t = g * TPG + j
                    nc.tensor.transpose(
                        out=pdx_t[:, bass.ts(j, P)],
                        in_=dense_sb[:, bass.ts(t, DENSE_A)],
                        identity=ident,
                    )
                return pdx_t

            def dense_mm1(g, pdx_t):
                dxt = smallp.tile([DENSE_A, GROUP], fp16, tag="dxt")
                nc.vector.tensor_copy(out=dxt[:], in_=pdx_t[:])
                ph = psmallp.tile([H_BOT, GROUP], f32, tag="psmall")
                nc.tensor.matmul(out=ph[:], lhsT=w1a_sb, rhs=dxt[:], start=True, stop=True)
                return ph

            def dense_mm2(g, ph):
                ha = ha2[g % 2]
                nc.scalar.activation(
                    out=ha[0:H_BOT, :], in_=ph[:],
                    func=mybir.ActivationFunctionType.Relu,
                )
                pd = psmallp.tile([D, GROUP], f32, tag="psmall")
                nc.tensor.matmul(out=pd[:], lhsT=w2a_sb, rhs=ha[:], start=True, stop=True)
                nc.scalar.activation(
                    out=dta[g][0:D, :], in_=pd[:],
                    func=mybir.ActivationFunctionType.Copy,
                )

            # group 0+1 dense runs up front, under the first gather; the PE
            # warm-up matmuls engage the HAM clock boost (transposes don't
            # count as PE-busy for it)
            pdx0 = dense_tr(0)
            ph0 = dense_mm1(0, pdx0)
            pdx1 = dense_tr(1)
            dense_mm2(0, ph0)
            ph1 = dense_mm1(1, pdx1)
            dense_mm2(1, ph1)

            # ---- fp8-pair transposes + DoubleRow top-MLP accumulation ----
            tail = []  # deferred (o1 relu, tw2 matmul, sigmoid) of prev group

            def flush_tail():
                while tail:
                    tail.pop(0)()

            for g in range(N_G):
                po1 = po1p.tile([H_TOP, GROUP], f32, tag="po1")
                mms = []
                casted = []

                def emit_tchunks(dc, g=g, mms=mms, casted=casted, po1=po1):
                    # transposes for two k-chunks into one PSUM tile
                    cs = [2 * dc] + ([2 * dc + 1] if 2 * dc + 1 < 7 else [])
                    ptr_t = ptrp.tile([P, 2 * GROUP], bf16, tag="ptr")
                    for ci, c in enumerate(cs):
                        cw = CW[c]
                        for j in range(TPG):
                            o = j * W_T + c * 128
                            nc.tensor.transpose(
                                out=ptr_t[0:cw, ci * GROUP + j * P : ci * GROUP + (j + 1) * P],
                                in_=embs[g][:, o : o + cw].bitcast(bf16),
                                identity=identu,
                            )
                    casted.append((dc, cs, ptr_t))

                def emit_cast(g=g, mms=mms, casted=casted, po1=po1):
                    dc, cs, ptr_t = casted.pop(0)
                    embt = embtp.tile([P, 2 * GROUP], bf16, tag="embt")
                    wid = len(cs) * GROUP
                    if dc % 2 == 0:
                        nc.vector.tensor_copy(out=embt[:, 0:wid], in_=ptr_t[:, 0:wid])
                    else:
                        nc.scalar.activation(
                            out=embt[:, 0:wid], in_=ptr_t[:, 0:wid],
                            func=mybir.ActivationFunctionType.Copy,
                        )
                    for ci, c in enumerate(cs):
                        def mm(c=c, ci=ci, embt=embt):
                            cw = CW[c]
                            lhsT = tw1dr_sb[0:cw, c * 32 : (c + 1) * 32].rearrange(
                                "p (i m) -> p i m", i=2
                            )
                            rhs = embt[0:cw, bass.ts(ci, GROUP)].bitcast(fp8).rearrange(
                                "p (n i) -> p i n", i=2
                            )
                            nc.tensor.matmul(
                                out=po1[:], lhsT=lhsT, rhs=rhs,
                                start=(c == 0), stop=False,
                                perf_mode=mybir.MatmulPerfMode.DoubleRow,
                            )
                        mms.append(mm)

                # transpose phase (casts trail by one dchunk), then matmul phase
                emit_tchunks(0)
                emit_cast()
                emit_tchunks(1)
                if g < N_G - 2:
                    pdx_t = dense_tr(g + 2)
                emit_cast()
                flush_tail()
                emit_tchunks(2)
                emit_cast()
                mms.pop(0)()                      # MM(0)
                mms.pop(0)()                      # MM(1)
                if g < N_G - 2:
                    ph = dense_mm1(g + 2, pdx_t)
                emit_tchunks(3)                   # single chunk 6
                emit_cast()
                mms.pop(0)()                      # MM(2)
                mms.pop(0)()                      # MM(3)
                if g < N_G - 2:
                    dense_mm2(g + 2, ph)
                mms.pop(0)()                      # MM(4)
                mms.pop(0)()                      # MM(5)
                mms.pop(0)()                      # MM(6)
                nc.tensor.matmul(
                    out=po1[:], lhsT=tw1da_sb, rhs=dta[g][:], start=False, stop=True
                )

                def make_tail(g=g, po1=po1):
                    halves = 2 if g == N_G - 1 else 1
                    def run():
                        o1 = smallp.tile([H_TOP, GROUP], fp16, tag="o1")
                        w = GROUP // halves
                        plgs = []
                        for h in range(halves):
                            if h == 0 and halves == 2:
                                # ACT relu with fused descale, parallel to the
                                # DVE half below
                                nc.scalar.activation(
                                    out=o1[:, 0:w], in_=po1[:, 0:w],
                                    func=mybir.ActivationFunctionType.Relu,
                                    scale=1.0 / (FP8_SCALE * FP8_SCALE),
                                )
                            else:
                                nc.vector.tensor_scalar(
                                    out=o1[:, h * w : (h + 1) * w],
                                    in0=po1[:, h * w : (h + 1) * w],
                                    scalar1=1.0 / (FP8_SCALE * FP8_SCALE), scalar2=0.0,
                                    op0=mybir.AluOpType.mult, op1=mybir.AluOpType.max,
                                )
                            plg = psmallp.tile([1, GROUP], f32, tag="psmall")
                            nc.tensor.matmul(
                                out=plg[:, 0:w], lhsT=tw2_sb,
                                rhs=o1[:, h * w : (h + 1) * w], start=True, stop=True,
                            )
                            plgs.append(plg)
                        for h, plg in enumerate(plgs):
                            nc.scalar.activation(
                                out=y_row[:, g * GROUP + h * w : g * GROUP + (h + 1) * w],
                                in_=plg[:, 0:w],
                                func=mybir.ActivationFunctionType.Sigmoid,
                                bias=tb2_sb,
                            )
                        nc.sync.dma_start(
                            out=y_d[:, bass.ts(g, GROUP)],
                            in_=y_row[:, bass.ts(g, GROUP)],
                        )
                    return run

                tail.append(make_tail())
            flush_tail()

    nc.compile()
    return nc


_NC_CACHE = {}


def _get_nc():
    if "nc" not in _NC_CACHE:
        _NC_CACHE["nc"] = build_kernel()
    return _NC_CACHE["nc"]


FP8_NP = ml_dtypes.float8_e4m3  # what mybir.dt.float8e4 maps to


def make_in_maps(dense_x, sparse_x, tables, w1, b1, w2, b2, tw1, tb1, tw2, tb2):
    s = FP8_SCALE
    t8 = (np.asarray(tables, np.float32).reshape(F * CARD, D) * s).astype(FP8_NP)
    tables_u16 = np.ascontiguousarray(t8).view(np.uint16)  # [V, 32]
    comb = np.asarray(sparse_x).astype(np.int32) + (
        np.arange(F, dtype=np.int32) * CARD
    )[None, :]
    dense_f = np.asarray(dense_x).astype(np.float16)
    dense_aug = np.concatenate(
        [dense_f, np.ones((dense_f.shape[0], 1), np.float16)], axis=1
    )  # [B, 14] with ones column
    tw1 = np.asarray(tw1, np.float32)
    # tw1dr[p, c*32 + i*16 + m] = fp8(s * tw1[c*256 + 2p + i, m])
    tw1dr = np.zeros((P, 7 * 2 * H_TOP), dtype=FP8_NP)
    for c in range(7):
        rows = 2 * CW[c]
        blk = (tw1[c * 256 : c * 256 + rows] * s).astype(FP8_NP)  # [rows, 16]
        blk = blk.reshape(CW[c], 2, H_TOP).reshape(CW[c], 2 * H_TOP)
        tw1dr[0 : CW[c], c * 32 : (c + 1) * 32] = blk
    tw1da = np.vstack(
        [tw1[K_EMB : K_EMB + D], np.asarray(tb1, np.float32)[None, :]]
    ) * (s * s)
    blob = np.zeros((P, BLOB_W), dtype=np.float16)
    blob[:, 0:128] = np.eye(P, dtype=np.float16)
    blob[:, 128:256] = np.eye(P, dtype=ml_dtypes.bfloat16).view(np.uint16).view(np.float16)
    blob[:, 256:368] = tw1dr.view(np.uint16).view(np.float16)  # 224 fp8 -> 112 fp16
    blob[0:DENSE_A, 368:376] = np.vstack(
        [np.asarray(w1, np.float32), np.asarray(b1, np.float32)[None, :]]
    ).astype(np.float16)
    blob[0 : H_BOT + 1, 376:440] = np.vstack(
        [np.asarray(w2, np.float32), np.asarray(b2, np.float32)[None, :]]
    ).astype(np.float16)
    blob[0 : D + 1, 440:456] = tw1da.astype(np.float16)
    blob[0:H_TOP, 456:457] = np.asarray(tw2, np.float32).astype(np.float16)
    blob[0:1, 458:460] = (
        np.asarray(tb2, np.float32).reshape(1, 1).view(np.float16)
    )
    shared = {
        "tables": tables_u16,
        "blob": blob,
    }
    in_maps = []
    for c in range(N_CORES):
        m = dict(shared)
        # host pre-transpose: [p, (t f)] so the device DMA is contiguous
        dl = dense_aug[c * B_LOC : (c + 1) * B_LOC]
        m["dense_x"] = np.ascontiguousarray(
            dl.reshape(N_T, P, DENSE_A).transpose(1, 0, 2).reshape(P, N_T * DENSE_A)
        )
        cl = comb[c * B_LOC : (c + 1) * B_LOC]
        m["sparse_x"] = np.ascontiguousarray(
            cl.reshape(N_T, P, F).transpose(1, 0, 2).reshape(P, N_T * F)
        )
        in_maps.append(m)
    return in_maps


def kernel(**inputs):
    from concourse.bass_utils import run_bass_kernel_spmd

    nc = _get_nc()
    in_maps = make_in_maps(**inputs)
    res = run_bass_kernel_spmd(nc, in_maps, core_ids=list(range(N_CORES)))
    out = np.concatenate([r["y"].reshape(-1) for r in res.results])
    return out.reshape(B, 1).astype(np.float32)
